# revision 1
# baseline (speedup 1.0000x reference)
"""Trainium2 Bass kernel for nn_CrossAttentionBlock.

Reference computation (per batch b of 16):
    q  = einsum('chw,cp->hwp', x[b], Wq)      # (HW=4096, P=512)
    kt = einsum('nd,dp->pn',  y[b], Wk)       # (P, N=128)
    v  = y[b] @ Wv                            # (N, P)
    s  = (q @ kt) / sqrt(C)                   # (HW, N)
    a  = softmax(s, axis=HW)                  # over the SPATIAL axis
    o  = (a @ v) @ Wout                       # (HW, C)
    out = x + o.T.reshape(C, H, W)

Sharding: pure data-parallel over batch, 2 batches per core, no
collectives.

Per-core device algorithm (everything transposed so the softmax axis is
the SBUF free axis and x is consumed in its native (C, HW) layout).
Because N=128 << HW=4096, the two P=512-wide projections fold into small
per-batch matrices once per batch instead of once per pixel:
    ktb = Wk.T(chunks) @ yT          (P, N)    per batch
    vT  = Wv.T(chunks) @ yT          (P, N)
    M   = WqT.T @ ktb = Wq @ ktb     (C, N)    [Wq pre-scaled by 1/sqrt(C)]
    sT[:, j] = M.T @ x[:, j]         (N, hw-chunk)  <- only 4 matmuls/chunk
    aT = exp(sT) (PSUM->SBUF fused activation, accum_out gives row sums Z)
    VW  = (vT.T @ Wout) * (1/Z)[n]   (N, C)    normalization folded here
    out[:, j] = VW.T @ aT[:, j] + x[:, j]      <- 4 matmuls + add/chunk

The exp needs no max subtraction: scores are ~N(0,1) by construction
(unit-variance inputs, 1/sqrt(fan-in)-scaled weights), so exp stays well
inside fp32 range; softmax is shift-invariant.

Matmuls run as float32r (4-byte fp32 data, full-rate PE mode for
moving-dim >= 256) with fp32 PSUM accumulation.
"""

import sys

sys.path.insert(0, "/opt/trn_rl_repo")

import numpy as np

import concourse.bass as bass
import concourse.mybir as mybir
import concourse.tile as tile
from concourse.vector_clock import ScopedClock

B, C, H, W = 16, 512, 64, 64
HW = H * W
N_COND, D_COND, P = 128, 1024, 512
N_CORES = 8
BPC = B // N_CORES  # batches per core

F32 = mybir.dt.float32
F32R = mybir.dt.float32r
AX = mybir.AxisListType.X
EXP = mybir.ActivationFunctionType.Exp

PC = C // 128   # 4 chunks over C
PP = P // 128   # 4 chunks over P
PD = D_COND // 128  # 8 chunks over D
NJ = HW // 512  # 8 hw chunks of 512
XW = 2048       # x DMA tile width (free dim)
NXJ = HW // XW  # 2 x-tiles per (b, cc)


class SplitDrainTileContext(tile.TileContext):
    """This walrus build accepts only one sem wait per CTRL/drain
    instruction; Tile's tail drain waits on the whole global clock.
    Split the waits across a chain of drains on SP."""

    MAX_WAITS = 1

    def _drain_and_barrier(self, tick_clock, wait_clock):
        drain_inst = self.nc.sync.drain()
        wait_clock.add_sem_waits(
            drain_inst.ins, ScopedClock({None: tick_clock.global_clock})
        )
        si = drain_inst.ins.sync_info
        if si is not None and si.on_wait and len(si.on_wait) > self.MAX_WAITS:
            waits = list(si.on_wait)
            drain_inst.ins.sync_info = mybir.SyncInfo(
                on_wait=waits[: self.MAX_WAITS],
                on_update=list(si.on_update or []),
            )
            for i in range(self.MAX_WAITS, len(waits), self.MAX_WAITS):
                extra = self.nc.sync.drain()
                extra.ins.sync_info = mybir.SyncInfo(
                    on_wait=waits[i : i + self.MAX_WAITS], on_update=[]
                )
        self.nc.all_engine_barrier()
        assert self.sems is not None
        popped = self.nc._tile_sem_poison_stack.pop()
        assert popped is self._sem_poison
        self.nc.clear_and_free_semaphores(list(self.sems.allocated().values()))
        self.nc.all_engine_barrier()


def r(ap):
    """View an fp32 AP as float32r for full-rate PE matmul."""
    return ap.bitcast(F32R)



def split_multi_waits(nc):
    """This walrus build's codegen accepts at most ONE sem wait per
    instruction (any struct type). Split extra waits onto same-engine
    NoOps inserted immediately before the instruction."""
    ctr = [0]
    for fn in nc.m.functions:
        for bb in fn.blocks:
            insts = bb.instructions
            new = []
            changed = False
            for inst in insts:
                si = inst.sync_info
                if si is not None and si.on_wait and len(si.on_wait) > 1:
                    waits = list(si.on_wait)
                    for w in waits[:-1]:
                        nop = mybir.InstNoOp(
                            name=f"I-wsplit-{ctr[0]}", ins=[], outs=[]
                        )
                        ctr[0] += 1
                        nop.engine = inst.engine
                        nop.sync_info = mybir.SyncInfo(on_wait=[w], on_update=[])
                        new.append(nop)
                    inst.sync_info = mybir.SyncInfo(
                        on_wait=[waits[-1]], on_update=list(si.on_update or [])
                    )
                    changed = True
                new.append(inst)
            if changed:
                bb.instructions = new


def build_nc(reps: int = 1, split_waits: bool = True) -> bass.Bass:
    nc = bass.Bass()

    xc = nc.declare_dram_parameter("xc", [BPC, C, HW], F32, isOutput=False)
    ytc = nc.declare_dram_parameter("ytc", [BPC, D_COND, N_COND], F32, isOutput=False)
    wqt = nc.declare_dram_parameter("wqt", [P, C], F32, isOutput=False)
    wk = nc.declare_dram_parameter("wk", [D_COND, P], F32, isOutput=False)
    wv = nc.declare_dram_parameter("wv", [D_COND, P], F32, isOutput=False)
    wo = nc.declare_dram_parameter("wo", [P, C], F32, isOutput=False)
    outc = nc.declare_dram_parameter("outc", [BPC, C, HW], F32, isOutput=True)

    with SplitDrainTileContext(nc) as tc:
        with (
            tc.tile_pool(name="persist", bufs=1) as persist,
            tc.tile_pool(name="attn", bufs=BPC) as attn_pool,
            tc.tile_pool(name="outsb", bufs=8) as out_pool,
            tc.tile_pool(name="stats", bufs=2) as stats,
            tc.tile_pool(name="ps_pre", bufs=2, space="PSUM") as ps_pre,
            tc.tile_pool(name="ps_s", bufs=3, space="PSUM") as ps_s,
            tc.tile_pool(name="ps_o", bufs=3, space="PSUM") as ps_o,
        ):
            for rep in range(reps):
                # ---- per-rep persistent small tensors ----
                wo_sb = [persist.tile([128, C], F32, tag=f"wo{i}", name=f"wo_sb{i}") for i in range(PP)]
                kt_sb = [persist.tile([128, BPC * 128], F32, tag=f"kt{i}", name=f"kt_sb{i}") for i in range(PP)]
                vt_sb = [persist.tile([128, BPC * 128], F32, tag=f"vt{i}", name=f"vt_sb{i}") for i in range(PP)]
                m_sb = [persist.tile([128, BPC * 128], F32, tag=f"m{i}", name=f"m_sb{i}") for i in range(PC)]
                vw_sb = [persist.tile([128, C], F32, tag=f"vw{i}", name=f"vw_sb{i}") for i in range(BPC)]

                # ---- preamble: yT, kt, vT, M (weight pools released after) ----
                with (
                    tc.tile_pool(name="pre_w", bufs=1) as pre_w,
                    tc.tile_pool(name="pre_y", bufs=1) as pre_y,
                ):
                    yt_sb = [
                        pre_y.tile([128, BPC * 128], F32, tag=f"yt{i}", name=f"yt_sb{i}") for i in range(PD)
                    ]
                    for dc in range(PD):
                        for b in range(BPC):
                            nc.sync.dma_start(
                                out=r(yt_sb[dc][:, b * 128 : (b + 1) * 128]),
                                in_=r(ytc[b, dc * 128 : (dc + 1) * 128, :]),
                            )
                    wk_sb = [pre_w.tile([128, P], F32, tag=f"wk{i}", name=f"wk_sb{i}") for i in range(PD)]
                    wv_sb = [pre_w.tile([128, P], F32, tag=f"wv{i}", name=f"wv_sb{i}") for i in range(PD)]
                    wqt_sb = [pre_w.tile([128, C], F32, tag=f"wqt{i}", name=f"wqt_sb{i}") for i in range(PP)]
                    for dc in range(PD):
                        nc.sync.dma_start(out=r(wk_sb[dc]), in_=r(wk[dc * 128 : (dc + 1) * 128, :]))
                    for pc in range(PP):
                        nc.sync.dma_start(out=r(wqt_sb[pc]), in_=r(wqt[pc * 128 : (pc + 1) * 128, :]))
                    for dc in range(PD):
                        nc.sync.dma_start(out=r(wv_sb[dc]), in_=r(wv[dc * 128 : (dc + 1) * 128, :]))
                    for pc in range(PP):
                        nc.sync.dma_start(out=r(wo_sb[pc]), in_=r(wo[pc * 128 : (pc + 1) * 128, :]))

                    # kt[pc] = sum_dc Wk[dc, pc].T @ yT[dc]  -> (128p, BPC*128n)
                    for pc in range(PP):
                        ps = ps_pre.tile([128, C], F32, tag="pre")
                        pss = ps[:, : BPC * 128]
                        for dc in range(PD):
                            nc.tensor.matmul(
                                pss,
                                r(wk_sb[dc][:, pc * 128 : (pc + 1) * 128]),
                                r(yt_sb[dc]),
                                start=(dc == 0),
                                stop=(dc == PD - 1),
                            )
                        nc.vector.tensor_copy(r(kt_sb[pc]), pss)
                    # vT[pc] = sum_dc Wv[dc, pc].T @ yT[dc]
                    for pc in range(PP):
                        ps = ps_pre.tile([128, C], F32, tag="pre")
                        pss = ps[:, : BPC * 128]
                        for dc in range(PD):
                            nc.tensor.matmul(
                                pss,
                                r(wv_sb[dc][:, pc * 128 : (pc + 1) * 128]),
                                r(yt_sb[dc]),
                                start=(dc == 0),
                                stop=(dc == PD - 1),
                            )
                        nc.vector.tensor_copy(r(vt_sb[pc]), pss)
                    # M[cc] = sum_pc WqT[pc, cc].T @ kt[pc]  -> (128c, BPC*128n)
                    for cc in range(PC):
                        ps = ps_pre.tile([128, C], F32, tag="pre")
                        pss = ps[:, : BPC * 128]
                        for pc in range(PP):
                            nc.tensor.matmul(
                                pss,
                                r(wqt_sb[pc][:, cc * 128 : (cc + 1) * 128]),
                                r(kt_sb[pc]),
                                start=(pc == 0),
                                stop=(pc == PP - 1),
                            )
                        nc.vector.tensor_copy(m_sb[cc], pss)

                # ---- main phase: per-batch pipelines (scheduler
                # overlaps b0 stores with b1 loads/compute) ----
                with tc.tile_pool(name="xtiles", bufs=BPC * PC * NXJ) as x_pool:
                    x_sb = {}
                    for b in range(BPC):
                        for cc in range(PC):
                            for xj in range(NXJ):
                                t = x_pool.tile([128, XW], F32, tag="x", name=f"x{b}_{cc}_{xj}")
                                nc.sync.dma_start(
                                    out=t,
                                    in_=xc[
                                        b,
                                        cc * 128 : (cc + 1) * 128,
                                        xj * XW : (xj + 1) * XW,
                                    ],
                                )
                                x_sb[(b, cc, xj)] = t
                    for b in range(BPC):
                        # pass 1: scores + fused exp
                        at_b = attn_pool.tile([128, HW], F32, tag="at", name=f"at{b}")
                        part_b = stats.tile([128, NJ], F32, tag="part", name=f"part{b}")
                        for j in range(NJ):
                            xj, xo = divmod(j * 512, XW)
                            ps = ps_s.tile([128, 512], F32, tag="s")
                            for cc in range(PC):
                                nc.tensor.matmul(
                                    ps,
                                    m_sb[cc][:, b * 128 : (b + 1) * 128],
                                    x_sb[(b, cc, xj)][:, xo : xo + 512],
                                    start=(cc == 0),
                                    stop=(cc == PC - 1),
                                )
                            nc.scalar.activation(
                                out=r(at_b[:, j * 512 : (j + 1) * 512]),
                                in_=ps,
                                func=EXP,
                                accum_out=part_b[:, j : j + 1],
                            )
                        # softmax normalizer folded into VW = (vT.T @ Wout)/Z
                        zsum = stats.tile([128, 1], F32, tag="z", name=f"z{b}")
                        nc.vector.reduce_sum(out=zsum, in_=part_b, axis=AX)
                        rz = stats.tile([128, 1], F32, tag="rz", name=f"rz{b}")
                        nc.vector.reciprocal(out=rz, in_=zsum)
                        ps = ps_pre.tile([128, C], F32, tag="pre")
                        for pc in range(PP):
                            nc.tensor.matmul(
                                ps,
                                r(vt_sb[pc][:, b * 128 : (b + 1) * 128]),
                                r(wo_sb[pc]),
                                start=(pc == 0),
                                stop=(pc == PP - 1),
                            )
                        nc.vector.tensor_scalar_mul(r(vw_sb[b]), ps, rz)
                        # pass 2: out = VW.T @ aT + x
                        for j in range(NJ):
                            xj, xo = divmod(j * 512, XW)
                            for cc in range(PC):
                                ps = ps_o.tile([128, 512], F32, tag="o")
                                nc.tensor.matmul(
                                    ps,
                                    r(vw_sb[b][:, cc * 128 : (cc + 1) * 128]),
                                    r(at_b[:, j * 512 : (j + 1) * 512]),
                                    start=True,
                                    stop=True,
                                )
                                o_sb = out_pool.tile([128, 512], F32, tag="o_sb")
                                nc.vector.tensor_add(
                                    o_sb, ps, x_sb[(b, cc, xj)][:, xo : xo + 512]
                                )
                                nc.sync.dma_start(
                                    out=outc[
                                        b,
                                        cc * 128 : (cc + 1) * 128,
                                        j * 512 : (j + 1) * 512,
                                    ],
                                    in_=o_sb,
                                )
    if split_waits:
        split_multi_waits(nc)
    return nc


def shard_inputs(x, y, Wq, Wk, Wv, Wout):
    """Host-side: fold 1/sqrt(C) into Wq, pre-transpose Wq and y, shard
    x/y by batch."""
    scale = np.float32(1.0 / np.sqrt(C))
    wqt = np.ascontiguousarray((np.asarray(Wq) * scale).T.astype(np.float32))
    wk = np.ascontiguousarray(np.asarray(Wk, dtype=np.float32))
    wv = np.ascontiguousarray(np.asarray(Wv, dtype=np.float32))
    wo = np.ascontiguousarray(np.asarray(Wout, dtype=np.float32))
    x_r = np.asarray(x, dtype=np.float32).reshape(B, C, HW)
    y_t = np.ascontiguousarray(np.asarray(y, dtype=np.float32).transpose(0, 2, 1))
    in_maps = []
    for core in range(N_CORES):
        b0 = core * BPC
        in_maps.append(
            {
                "xc": np.ascontiguousarray(x_r[b0 : b0 + BPC]),
                "ytc": np.ascontiguousarray(y_t[b0 : b0 + BPC]),
                "wqt": wqt,
                "wk": wk,
                "wv": wv,
                "wo": wo,
            }
        )
    return in_maps


def kernel(x, y, Wq, Wk, Wv, Wout):
    from concourse.bass_utils import run_bass_kernel_spmd

    nc = build_nc(reps=1)
    in_maps = shard_inputs(x, y, Wq, Wk, Wv, Wout)
    res = run_bass_kernel_spmd(nc, in_maps, list(range(N_CORES)))
    out = np.empty((B, C, HW), dtype=np.float32)
    for core in range(N_CORES):
        b0 = core * BPC
        out[b0 : b0 + BPC] = res.results[core]["outc"]
    return out.reshape(B, C, H, W)

